# revision 4
# baseline (speedup 1.0000x reference)
"""Trainium2 Bass kernel for ContrastiveLossWithDiffusion.

Strategy (8 NeuronCores, SPMD, row-sharded):
  - reference:  A = dense 0/1 adjacency (dups collapsed), diffusion = D^-1/2 A D^-1/2,
                x <- diffusion^10 @ x, then log-sigmoid contrastive loss over edges.
  - algebra:    diffusion^10 x = D^1/2 (D^-1 A)^10 D^-1/2 x, so the loop only needs
                the 0/1 adjacency and cheap per-row scalings.
  - device:     each core owns 1024 rows. A^T column-slice (8192 x 1024, bf16 0/1)
                is built on device by GPSIMD local_scatter into resident SBUF and
                reused for all 10 matmul steps (no HBM re-reads).  Per step:
                psum[c,i] += sum_j w[j,c] * AT[j,i] (w stationary bf16), scale by
                1/deg, AllGather bf16 embeddings, reload via DMA-transpose.
  - loss:       pos/neg pair dots via GPSIMD dma_gather of final-embedding rows from
                DRAM + DVE multiply/reduce + ScalarE sigmoid/ln, AllReduce partials.
Host does only integer index preprocessing (dedup, degree counts, index layouts).
"""

import sys

for _p in ("/opt/trn_rl_repo",):
    if _p not in sys.path:
        sys.path.insert(0, _p)

import numpy as np

# problem constants (hardcoded per spec)
N = 8192
C = 128
NCORES = 8
L = N // NCORES            # rows per core = 1024
JT = N // 128              # 64 j-tiles
TI = L // 128              # 8 i-tiles per core
STEPS = 10
E = 262144
EC = E // NCORES           # 32768 pos edges per core
CH = 1024                  # pos-pair chunk size
NCH = EC // CH             # 16 chunks
INV_T = 2.0                # 1 / temperature


def _build_program(K):
    """Build the SPMD Bass/Tile program. K = max scatter indices per (dst,partition)."""
    from concourse import bass, bacc, tile, mybir

    f32 = mybir.dt.float32
    bf16 = mybir.dt.bfloat16
    i16 = mybir.dt.int16
    AF = mybir.ActivationFunctionType
    ALU = mybir.AluOpType
    AX = mybir.AxisListType

    nc = bacc.Bacc("TRN2", target_bir_lowering=False, debug=False,
                   num_devices=NCORES)

    # ---- I/O ----
    xT_d = nc.declare_dram_parameter("xT", [C, L], f32, isOutput=False)
    degpt_d = nc.declare_dram_parameter("deg_pt", [128, TI], f32, isOutput=False)
    degrow_d = nc.declare_dram_parameter("deg_row", [1, L], f32, isOutput=False)
    aidx_d = nc.declare_dram_parameter("a_idx", [128, JT, K], i16, isOutput=False)
    poss_d = nc.declare_dram_parameter("pos_s", [128, NCH, CH // 16], i16, isOutput=False)
    posd_d = nc.declare_dram_parameter("pos_d", [128, NCH, CH // 16], i16, isOutput=False)
    neg_d = nc.declare_dram_parameter("neg_idx", [128, L // 16], i16, isOutput=False)
    ident_d = nc.declare_dram_parameter("identity", [128, 128], f32, isOutput=False)
    loss_d = nc.declare_dram_parameter("loss", [1, 1], f32, isOutput=True)

    rg = [list(range(NCORES))]

    with tile.TileContext(nc, num_cores=NCORES) as tc:
        with (
            tc.tile_pool(name="big", bufs=1) as big,
            tc.tile_pool(name="wt", bufs=2) as wtp,
            tc.tile_pool(name="gp", bufs=2) as gp,
            tc.tile_pool(name="psA", bufs=2, space="PSUM") as psA,
            tc.tile_pool(name="psS", bufs=2, space="PSUM") as psS,
            tc.tile_pool(name="dloop", bufs=2, space="DRAM") as dloop,
            tc.tile_pool(name="dfix", bufs=1, space="DRAM") as dfix,
        ):
            # ---- persistent SBUF ----
            AT = big.tile([128, JT, L], bf16)           # A^T slice, 0/1
            w = big.tile([128, JT, C], bf16)            # full graph embeddings (j,c)
            xT_s = big.tile([128, L], f32, tag="xbuf")
            aidx_s = big.tile([128, JT, K], i16)
            neg_s = big.tile([128, L // 16], i16)
            ident_s = big.tile([128, 128], f32)
            ones_sc = big.tile([128, K], bf16)
            ones_row = big.tile([1, 128], f32)
            degpt_s = big.tile([128, TI], f32)
            degrow_s = big.tile([1, L], f32)
            dinv_bc = big.tile([128, L], f32)           # 1/deg broadcast rows
            dis_bc = big.tile([128, L], f32)            # deg^-1/2 broadcast rows
            v_loc = big.tile([128, TI, C], bf16)        # my final embedding rows
            pos_acc = big.tile([128, NCH], f32)
            neg_acc = big.tile([128, 1], f32)
            T0 = big.tile([128, L], f32, tag="xbuf")

            # ---- input loads ----
            nc.sync.dma_start(xT_s[:], xT_d[:])
            nc.sync.dma_start(aidx_s[:], aidx_d[:])
            nc.sync.dma_start(neg_s[:], neg_d[:])
            nc.sync.dma_start(ident_s[:], ident_d[:])
            nc.sync.dma_start(degpt_s[:], degpt_d[:])
            nc.sync.dma_start(degrow_s[:], degrow_d[:])
            nc.vector.memset(ones_sc[:], 1.0)
            nc.vector.memset(ones_row[:], 1.0)

            # ---- build A^T slice by per-partition scatter of ones ----
            for jt in range(JT):
                nc.gpsimd.local_scatter(
                    AT[:, jt, :], ones_sc[:], aidx_s[:, jt, :],
                    channels=128, num_elems=L, num_idxs=K,
                )

            # ---- degree -> 1/deg (HW divide) and deg^-1/2 (sqrt + Newton) ----
            def rsqrt_newton(dst, deg, dinv, shape):
                y0 = big.tile(shape, f32, tag="dg0")
                t1 = big.tile(shape, f32, tag="dg1")
                nc.vector.reciprocal(dinv[:], deg[:])
                nc.scalar.activation(y0[:], dinv[:], AF.Sqrt)
                # Newton for deg^-1/2: y1 = y0 * (1.5 - 0.5 * deg * y0^2)
                nc.vector.tensor_tensor(t1[:], y0[:], y0[:], op=ALU.mult)
                nc.vector.tensor_tensor(t1[:], t1[:], deg[:], op=ALU.mult)
                nc.vector.tensor_scalar(t1[:], t1[:], -0.5, 1.5, op0=ALU.mult,
                                        op1=ALU.add)
                nc.vector.tensor_tensor(dst[:], y0[:], t1[:], op=ALU.mult)

            dis_pt = big.tile([128, TI], f32)
            dinv_pt = big.tile([128, TI], f32)
            rsqrt_newton(dis_pt, degpt_s, dinv_pt, [128, TI])
            dis_row = big.tile([1, L], f32)
            dinv_row = big.tile([1, L], f32)
            rsqrt_newton(dis_row, degrow_s, dinv_row, [1, L])

            # broadcast rows across partitions via rank-1 matmul (ones ⊗ row)
            MC = min(512, L)
            def bcast(dst, row):
                pb = psA.tile([128, L], f32, tag="ps")
                for h in range(0, L, MC):
                    nc.tensor.matmul(pb[:, h:h + MC], ones_row[:],
                                     row[:, h:h + MC], start=True, stop=True)
                nc.vector.tensor_copy(dst[:], pb[:])

            bcast(dinv_bc, dinv_row)
            bcast(dis_bc, dis_row)

            # ---- prologue: w0^T = xT * d^-1/2 (bf16), AG, load w tiles ----
            def ag_and_load(wT_tile):
                b_in = dloop.tile([128, L], bf16, tag="bin")
                b_out = dloop.tile([NCORES, 128, L], bf16, tag="bout")
                nc.sync.dma_start(b_in[:], wT_tile[:])
                nc.gpsimd.collective_compute(
                    "AllGather", mybir.AluOpType.bypass, replica_groups=rg,
                    ins=[b_in[:].opt()], outs=[b_out[:].opt()],
                )
                for r in range(NCORES):
                    nc.sync.dma_start_transpose(w[:, r * TI:(r + 1) * TI, :],
                                                b_out[r])

            wT0 = wtp.tile([128, L], bf16, tag="wT")
            nc.vector.tensor_tensor(wT0[:], xT_s[:], dis_bc[:], op=ALU.mult)
            ag_and_load(wT0)

            # ---- diffusion steps ----
            for t in range(1, STEPS + 1):
                ps = psA.tile([128, L], f32, tag="ps")
                for jt in range(JT):
                    st = jt == 0
                    sp = jt == JT - 1
                    for h in range(0, L, MC):
                        nc.tensor.matmul(ps[:, h:h + MC], w[:, jt, :],
                                         AT[:, jt, h:h + MC], start=st, stop=sp)
                if t < STEPS:
                    wT = wtp.tile([128, L], bf16, tag="wT")
                    nc.vector.tensor_tensor(wT[:], ps[:], dinv_bc[:], op=ALU.mult)
                    ag_and_load(wT)
                else:
                    # final: v = deg^-1/2 * (A w9); produce row-major v for gathers
                    nc.vector.tensor_tensor(T0[:], ps[:], dis_bc[:], op=ALU.mult)
                    for g in range(TI):
                        pt = psS.tile([128, 128], f32, tag="pt")
                        nc.tensor.transpose(pt[:], T0[:, g * 128:(g + 1) * 128],
                                            ident_s[:])
                        nc.vector.tensor_copy(v_loc[:, g, :], pt[:])
                    bv_in = dfix.tile([L, C], bf16)
                    bv_out = dfix.tile([N, C], bf16)
                    nc.sync.dma_start(
                        bv_in[:].rearrange("(g p) c -> p g c", p=128), v_loc[:])
                    nc.gpsimd.collective_compute(
                        "AllGather", mybir.AluOpType.bypass, replica_groups=rg,
                        ins=[bv_in[:].opt()], outs=[bv_out[:].opt()],
                    )

            # ---- pos / neg log-sigmoid sums ----
            def load_idx(dram_ap, tag):
                it = gp.tile([128, dram_ap.shape[-1]], i16, tag=tag)
                nc.sync.dma_start(it[:], dram_ap)
                return it

            def pair_block(idx_a, idx_b, nidx, groups, sig_scale, acc_ap, a_tile=None):
                if a_tile is None:
                    ia = load_idx(idx_a, "ia")
                    ga = gp.tile([128, groups, C], bf16, tag="ga")
                    nc.gpsimd.dma_gather(ga[:], bv_out[:], ia[:], num_idxs=nidx,
                                         num_idxs_reg=nidx, elem_size=C)
                else:
                    ga = a_tile
                ib = load_idx(idx_b, "ib")
                gb = gp.tile([128, groups, C], bf16, tag="gb")
                nc.gpsimd.dma_gather(gb[:], bv_out[:], ib[:], num_idxs=nidx,
                                     num_idxs_reg=nidx, elem_size=C)
                pr = gp.tile([128, groups, C], bf16, tag="pr")
                nc.vector.tensor_tensor(pr[:], ga[:], gb[:], op=ALU.mult)
                dots = gp.tile([128, groups], f32, tag="dots")
                nc.vector.tensor_reduce(dots[:], pr[:], axis=AX.X, op=ALU.add)
                sg = gp.tile([128, groups], f32, tag="sg")
                nc.scalar.activation(sg[:], dots[:], AF.Sigmoid, scale=sig_scale)
                lnv = gp.tile([128, groups], f32, tag="lnv")
                nc.scalar.activation(lnv[:], sg[:], AF.Ln, accum_out=acc_ap)

            for ch in range(NCH):
                pair_block(poss_d[:, ch, :], posd_d[:, ch, :], CH, CH // 128,
                           INV_T, pos_acc[:, ch:ch + 1])
            pair_block(None, neg_s[:], L, TI, -INV_T, neg_acc[:], a_tile=v_loc)

            # ---- combine, partition-reduce, AllReduce, output ----
            pos_sum = big.tile([128, 1], f32)
            nc.vector.tensor_reduce(pos_sum[:], pos_acc[:], axis=AX.X, op=ALU.add)
            part = big.tile([128, 1], f32)
            nc.vector.tensor_scalar(pos_sum[:], pos_sum[:], -1.0 / E, None,
                                    op0=ALU.mult)
            nc.vector.tensor_scalar(neg_acc[:], neg_acc[:], -1.0 / N, None,
                                    op0=ALU.mult)
            nc.vector.tensor_tensor(part[:], pos_sum[:], neg_acc[:], op=ALU.add)

            ones_col = big.tile([128, 1], f32)
            nc.vector.memset(ones_col[:], 1.0)
            pscalar = psS.tile([1, 1], f32, tag="psc")
            nc.tensor.matmul(pscalar[:], part[:], ones_col[:], start=True, stop=True)
            loss_sb = big.tile([1, 8], f32)
            nc.vector.memset(loss_sb[:], 0.0)
            nc.vector.tensor_copy(loss_sb[:, 0:1], pscalar[:])

            ar_in = dfix.tile([1, 8], f32)
            ar_out = dfix.tile([1, 8], f32)
            nc.sync.dma_start(ar_in[:], loss_sb[:])
            nc.gpsimd.collective_compute(
                "AllReduce", mybir.AluOpType.add, replica_groups=rg,
                ins=[ar_in[:].opt()], outs=[ar_out[:].opt()],
            )
            nc.sync.dma_start(loss_d[:], ar_out[0:1, 0:1])

    nc.compile()
    return nc


def _host_prep(embeddings, edge_index, rand_indices):
    """Integer index preprocessing + per-core input maps."""
    emb = np.asarray(embeddings, dtype=np.float32)
    src = np.asarray(edge_index[0], dtype=np.int64)
    dst = np.asarray(edge_index[1], dtype=np.int64)
    rnd = np.asarray(rand_indices, dtype=np.int64)

    keys = np.unique(src * N + dst)
    s_u = (keys // N).astype(np.int64)
    d_u = (keys % N).astype(np.int64)
    deg = np.bincount(s_u, minlength=N).astype(np.float32)

    # global max scatter count per (dst node, core) cell, padded even
    cell = np.bincount(d_u * NCORES + (s_u // L), minlength=N * NCORES)
    K = int(cell.max())
    K += K % 2
    K = max(K, 2)

    def wrap16(arr, reps=8):
        # position m -> [m % 16, m // 16], replicated across the 8 Q7 cores
        t = arr.reshape(-1, 16).T.astype(np.int16)
        return np.tile(t, (reps, 1))

    in_maps = []
    for c in range(NCORES):
        lo, hi = c * L, (c + 1) * L

        m = (s_u >= lo) & (s_u < hi)
        es = (s_u[m] - lo).astype(np.int16)
        ed = d_u[m]
        order = np.argsort(ed, kind="stable")
        ed_s = ed[order]
        es_s = es[order]
        cnt = np.bincount(ed_s, minlength=N)
        starts = np.concatenate([[0], np.cumsum(cnt)[:-1]])
        rank = np.arange(len(ed_s)) - starts[ed_s]
        a_idx = np.full((128, JT, K), -1, dtype=np.int16)
        a_idx[ed_s % 128, ed_s // 128, rank] = es_s

        pos_s = np.stack([wrap16(src[c * EC + i * CH:c * EC + (i + 1) * CH])
                          for i in range(NCH)], axis=1)
        pos_d = np.stack([wrap16(dst[c * EC + i * CH:c * EC + (i + 1) * CH])
                          for i in range(NCH)], axis=1)
        neg_idx = wrap16(rnd[lo:hi])

        in_maps.append({
            "xT": np.ascontiguousarray(emb[lo:hi].T),
            "deg_pt": np.ascontiguousarray(deg[lo:hi].reshape(TI, 128).T),
            "deg_row": deg[lo:hi].reshape(1, L),
            "a_idx": a_idx,
            "pos_s": np.ascontiguousarray(pos_s),
            "pos_d": np.ascontiguousarray(pos_d),
            "neg_idx": neg_idx,
            "identity": np.eye(128, dtype=np.float32),
        })
    return in_maps, K


_cache = {}


def kernel(embeddings, edge_index, rand_indices):
    from concourse.bass_utils import run_bass_kernel_spmd

    in_maps, K = _host_prep(embeddings, edge_index, rand_indices)
    if K not in _cache:
        _cache[K] = _build_program(K)
    nc = _cache[K]
    res = run_bass_kernel_spmd(nc, in_maps, list(range(NCORES)))
    return np.float32(res.results[0]["loss"][0, 0])
